# revision 14
# baseline (speedup 1.0000x reference)
# MoE kernel for Trainium2 (8 NeuronCores, expert-parallel).
#
# Strategy:
#  - Host: gate logits = x @ gate_w, top-2 + softmax, gather tokens per expert
#    (the "all-to-all by routed expert" from the sharding hint, done host-side
#    since we hold full inputs), pad each expert's token set to a common
#    capacity C = max expert load (token dim is the matmul free axis, so no
#    128-padding needed).
#  - Device (core e = expert e): h = gelu(x^T GEMM w1); y = h GEMM w2, both
#    bf16 on the PE array, fp32 PSUM accumulate. Loop order is f-major with
#    the token tile innermost; w2 is DMA'd into the SBUF space each w1 group
#    vacates, so the full h (C x DFF bf16) stays resident between the GEMMs.
#    w1 streams per f-tile (256KB) so the first matmul starts ~3us in.
#  - Host: scatter-add wts * (y + b2[e]) back into the output.
import math
from contextlib import ExitStack

import ml_dtypes
import numpy as np

import concourse.bass as bass
import concourse.mybir as mybir
import concourse.tile as tile
from concourse.bass_utils import run_bass_kernel_spmd

D = 1024
DFF = 4096
E = 8
TOP_K = 2
P = 128
KD = D // P      # 8  contraction tiles for GEMM1
NF = DFF // P    # 32 dff tiles (GEMM1 out / GEMM2 contraction)
ND = D // P      # 8  GEMM2 out tiles
T_TILE = 512
FG = 4           # f-tiles per SBUF weight-group slot
NG = NF // FG    # 8 such groups

BF16 = mybir.dt.bfloat16
F32 = mybir.dt.float32
NP_BF16 = np.dtype(ml_dtypes.bfloat16)

V2_MAX_C = 1152  # up to 4 token tiles (PSUM: 4 tags x 2 bufs = 8 banks)

_neff_cache = {}


def _split_multiwait_json(bir_bytes: bytes) -> bytes:
    """The walrus build in this container rejects instructions carrying more
    than one sync wait (or update). Split extras onto adjacent single-wait
    EventSemaphore carriers on the same engine: program order on the engine
    preserves the semantics exactly."""
    import json as _json

    bir = _json.loads(bir_bytes)
    for fn in bir["functions"]:
        for blk in fn["blocks"]:
            insts = blk.get("instructions", [])
            out = []
            for inst in insts:
                si = inst.get("sync_info")
                if si:
                    waits = si.get("on_wait") or []
                    if len(waits) > 1:
                        for i, w in enumerate(waits[:-1]):
                            out.append({
                                "debug": inst.get("debug", 0),
                                "engine": inst["engine"],
                                "ins": [],
                                "name": f"{inst['name']}_w{i}",
                                "opcode": "EventSemaphore",
                                "outs": [],
                                "sync_info": {"on_update": [], "on_wait": [w]},
                            })
                        si["on_wait"] = [waits[-1]]
                out.append(inst)
                if si:
                    ups = si.get("on_update") or []
                    if len(ups) > 1:
                        for i, u in enumerate(ups[1:]):
                            out.append({
                                "debug": inst.get("debug", 0),
                                "engine": inst["engine"],
                                "ins": [],
                                "name": f"{inst['name']}_u{i}",
                                "opcode": "EventSemaphore",
                                "outs": [],
                                "sync_info": {"on_update": [u], "on_wait": []},
                            })
                        si["on_update"] = [ups[0]]
            blk["instructions"] = out
    return _json.dumps(bir).encode()


def _patch_to_json(nc: bass.Bass) -> bass.Bass:
    orig = nc.to_json_bytes
    nc.to_json_bytes = lambda: _split_multiwait_json(orig())
    return nc


def _t_tiles_v2(C):
    """First tile small (fast compute start on a small x DMA), then 512s,
    with a small final tile so the last PSUM->SBUF->DRAM drain is short.
    At most 4 tiles (PSUM: 4 tags x 2 bufs = 8 banks)."""
    out, t0 = [], 0
    first = min(128, C)
    out.append((0, first))
    t0 = first
    while t0 < C:
        tsz = min(T_TILE, C - t0)
        out.append((t0, tsz))
        t0 += tsz
    last = out[-1][1]
    if len(out) < 4 and last > 192:
        o = out.pop()[0]
        out.append((o, last - 64))
        out.append((o + last - 64, 64))
    return out


def _build_bass_v2(C: int) -> bass.Bass:
    """One expert's MLP, f-major with token tiles innermost.

    DRAM layouts (host pre-blocked, every DMA chunk contiguous per partition):
      xs : [P, KD * C]   bf16; tile ti at cols KD*off, within it k*tsz + c,
                         row p holds x[k*128+p, off+c]
      w1 : [NG*P, FG*KD*P] bf16; row g*P+p, col j*KD*P + k*P + m holds
                         w1[k*128+p, (4g+j)*128+m]  (f-single = 2KB/row chunk)
      w2 : [DFF, D]      bf16 (natural layout; f-tile f at rows f*P)
      b1 : [P, NF]       f32  (pre-transposed: b1[p, f] = bias[f*128+p])
    Output:
      y  : [D, C]        f32; y[dd*128+p, off+c]
    """
    nc = bass.Bass()
    tt = _t_tiles_v2(C)
    n_t = len(tt)
    xs_h = nc.dram_tensor("xs", [P, KD * C], BF16, kind="ExternalInput")
    w1_h = nc.dram_tensor("w1", [NG * P, FG * KD * P], BF16, kind="ExternalInput")
    b1_h = nc.dram_tensor("b1", [P, NF], F32, kind="ExternalInput")
    w2_h = nc.dram_tensor("w2", [DFF, D], BF16, kind="ExternalInput")
    y_h = nc.dram_tensor("y", [D, C], F32, kind="ExternalOutput")

    with ExitStack() as ctx:
        tc = ctx.enter_context(tile.TileContext(nc))
        wpool = ctx.enter_context(tc.tile_pool(name="w", bufs=1))
        xpool = ctx.enter_context(tc.tile_pool(name="x", bufs=1))
        hpool = ctx.enter_context(tc.tile_pool(name="h", bufs=1))
        bpool = ctx.enter_context(tc.tile_pool(name="b", bufs=1))
        ypool = ctx.enter_context(tc.tile_pool(name="y", bufs=2))
        pspool = ctx.enter_context(tc.tile_pool(name="ps", bufs=2, space="PSUM"))

        # sync HWDGE ring: x tiles (t0 first: it gates the first matmul),
        # b1, then the other tiles' k-halves interleaved so the f0 k-loop
        # unblocks as early as possible, then (emitted during G1) the w2
        # groups as w1 slots free up.
        x_t = [xpool.tile([P, KD, tsz], BF16, tag=f"x{ti}", name=f"x{ti}")
               for ti, (off, tsz) in enumerate(tt)]
        KH = KD // 2

        def issue_x(ti, half):
            off, tsz = tt[ti]
            k0 = half * KH
            nc.sync.dma_start(
                x_t[ti][:, k0:k0 + KH, :],
                xs_h[:, KD * off + k0 * tsz:KD * off + (k0 + KH) * tsz]
                .rearrange("p (k c) -> p k c", k=KH),
            )

        issue_x(0, 0)
        issue_x(0, 1)
        b1_raw = bpool.tile([P, NF], F32, tag="b1r", name="b1r")
        nc.sync.dma_start(b1_raw[:], b1_h[:])
        for half in range(2):
            for ti in range(1, n_t):
                issue_x(ti, half)

        # scalar HWDGE ring: w1 f-singles, prefetch distance 4 ahead of G1.
        wg1 = [wpool.tile([P, FG, KD * P], BF16, tag=f"wg{g}", name=f"w1g{g}")
               for g in range(NG)]

        def issue_w1(f):
            g, j = divmod(f, FG)
            nc.scalar.dma_start(
                wg1[g][:, j, :],
                w1_h[g * P:(g + 1) * P, j * KD * P:(j + 1) * KD * P],
            )

        PREF = 4
        for f in range(min(PREF, NF)):
            issue_w1(f)
        # Funnel b1 through an ACT-engine copy: downstream gelus then reach it
        # via same-engine program order instead of an extra semaphore wait.
        b1_t = bpool.tile([P, NF], F32, tag="b1", name="b1")
        nc.scalar.copy(b1_t[:], b1_raw[:])

        gelu = mybir.ActivationFunctionType.Gelu
        wg2 = [None] * NG
        h_t = [[None] * n_t for _ in range(NF)]

        # GEMM1 + gelu, f-major, token tile innermost (stationary w reused).
        for f in range(NF):
            g, j = divmod(f, FG)
            pts = [pspool.tile([P, tsz], F32, tag=f"ps{ti}", name=f"p1_{f}_{ti}")
                   for ti, (off, tsz) in enumerate(tt)]
            for k in range(KD):
                for ti, (off, tsz) in enumerate(tt):
                    nc.tensor.matmul(
                        pts[ti][:],
                        wg1[g][:, j, k * P:(k + 1) * P],
                        x_t[ti][:, k, :],
                        start=(k == 0),
                        stop=(k == KD - 1),
                    )
            for ti, (off, tsz) in enumerate(tt):
                h = hpool.tile([P, tsz], BF16, tag=f"h{f}_{ti}", name=f"h{f}_{ti}")
                nc.scalar.activation(h[:], pts[ti][:], gelu, bias=b1_t[:, f:f + 1])
                h_t[f][ti] = h
            if f + PREF < NF:
                issue_w1(f + PREF)
            if j == FG - 1:
                # group g of w1 fully consumed -> stream w2's f-tiles into it
                wt = wpool.tile([P, FG, D], BF16, tag=f"wg{g}", name=f"w2g{g}")
                for jj in range(FG):
                    ff = g * FG + jj
                    nc.sync.dma_start(wt[:, jj, :], w2_h[ff * P:(ff + 1) * P, :])
                wg2[g] = wt

        # GEMM2, dd outer, f accumulation, token tile innermost.
        for dd in range(ND):
            pts = [pspool.tile([P, tsz], F32, tag=f"ps{ti}", name=f"p2_{dd}_{ti}")
                   for ti, (off, tsz) in enumerate(tt)]
            for f in range(NF):
                g, j = divmod(f, FG)
                for ti, (off, tsz) in enumerate(tt):
                    nc.tensor.matmul(
                        pts[ti][:],
                        wg2[g][:, j, dd * P:(dd + 1) * P],
                        h_t[f][ti][:],
                        start=(f == 0),
                        stop=(f == NF - 1),
                    )
            for ti, (off, tsz) in enumerate(tt):
                y_t = ypool.tile([P, tsz], F32, tag=f"y{ti}", name=f"y{dd}_{ti}")
                nc.vector.tensor_copy(y_t[:], pts[ti][:])
                nc.scalar.dma_start(y_h[dd * P:(dd + 1) * P, off:off + tsz], y_t[:])
    return _patch_to_json(nc)


# ---------------------------------------------------------------------------
# v3: pair-balanced expert-parallel. Experts sorted by load; big_i paired
# with small_i; the pair's two cores each run (half of big, half of small).
# Compiled chunk caps C_a >= C_b are maxes over cores. w1 streams as
# f-singles through 12 rotating SBUF slots; w2 streams as per-dd column
# slabs through 3 slots, so both experts' weights coexist cheaply.
# ---------------------------------------------------------------------------

V3_MAX_CA = 640  # chunk A: tiles (128, C_a-128), both <= 512
V3_MAX_CB = 576  # chunk B: tiles (C_b-64, 64)


def _t_tiles_v3a(C):
    if C <= 192:
        return [(0, C)]
    return [(0, 128), (128, C - 128)]


def _t_tiles_v3b(C):
    if C <= 128:
        return [(0, C)]
    return [(0, C - 64), (C - 64, 64)]


def _build_bass_v3(C_a: int, C_b: int) -> bass.Bass:
    nc = bass.Bass()
    tts = [_t_tiles_v3a(C_a), _t_tiles_v3b(C_b)]
    xbase = [0, KD * C_a]
    ybase = [0, C_a]
    xs_h = nc.dram_tensor("xs", [P, KD * (C_a + C_b)], BF16, kind="ExternalInput")
    w1_h = nc.dram_tensor("w1", [2 * NG * P, FG * KD * P], BF16, kind="ExternalInput")
    b1_h = nc.dram_tensor("b1", [P, 2 * NF], F32, kind="ExternalInput")
    w2_h = nc.dram_tensor("w2", [2 * ND * P, NF * P], BF16, kind="ExternalInput")
    y_h = nc.dram_tensor("y", [D, C_a + C_b], F32, kind="ExternalOutput")

    with ExitStack() as ctx:
        tc = ctx.enter_context(tile.TileContext(nc))
        wpool = ctx.enter_context(tc.tile_pool(name="w", bufs=1))
        spool = ctx.enter_context(tc.tile_pool(name="s", bufs=1))
        xpool = ctx.enter_context(tc.tile_pool(name="x", bufs=1))
        hpool = ctx.enter_context(tc.tile_pool(name="h", bufs=1))
        bpool = ctx.enter_context(tc.tile_pool(name="b", bufs=1))
        ypool = ctx.enter_context(tc.tile_pool(name="y", bufs=2))
        pspool = ctx.enter_context(tc.tile_pool(name="ps", bufs=2, space="PSUM"))

        # ---- sync ring: x (chunk A tile0 first), b1, then w2 slabs 0..2
        x_t = [[xpool.tile([P, KD, tsz], BF16, tag=f"x{c}_{ti}", name=f"x{c}_{ti}")
                for ti, (off, tsz) in enumerate(tts[c])] for c in range(2)]
        KH = KD // 2

        def issue_x(c, ti, k0, k1):
            off, tsz = tts[c][ti]
            nc.sync.dma_start(
                x_t[c][ti][:, k0:k1, :],
                xs_h[:, xbase[c] + KD * off + k0 * tsz:
                     xbase[c] + KD * off + k1 * tsz]
                .rearrange("p (k c) -> p k c", k=k1 - k0),
            )

        issue_x(0, 0, 0, KH)
        issue_x(0, 0, KH, KD)
        b1_raw = bpool.tile([P, 2 * NF], F32, tag="b1r", name="b1r")
        nc.sync.dma_start(b1_raw[:], b1_h[:])
        # chunk A's big tile in k-quarters (f0's k-loop unblocks sooner),
        # chunk B (needed only ~60us in) in k-halves
        for ti in range(1, len(tts[0])):
            for q in range(4):
                issue_x(0, ti, q * 2, q * 2 + 2)
        for half in range(2):
            for ti in range(len(tts[1])):
                issue_x(1, ti, half * KH, half * KH + KH)

        # w2 slabs stream in G2 processing order: chunk B's first, then A's
        NSLAB = 2 * ND
        SLAB_SEQ = [(1, dd) for dd in range(ND)] + [(0, dd) for dd in range(ND)]
        slab_t = {}

        def issue_slab(pos):
            c, dd = SLAB_SEQ[pos]
            i = c * ND + dd
            t = spool.tile([P, NF * P], BF16, tag=f"w2s{pos % 3}", name=f"w2s{i}")
            nc.sync.dma_start(t[:], w2_h[i * P:(i + 1) * P, :])
            slab_t[(c, dd)] = t

        # ---- scalar ring: w1 f-singles, 12 rotating slots, prefetch 6
        NW1 = 2 * NF
        w1_t = [None] * NW1

        def issue_w1(i):
            c, f = divmod(i, NF)
            g, j = divmod(f, FG)
            t = wpool.tile([P, KD * P], BF16, tag=f"w1s{i % 12}", name=f"w1s{i}")
            nc.scalar.dma_start(
                t[:],
                w1_h[(c * NG + g) * P:(c * NG + g + 1) * P,
                     j * KD * P:(j + 1) * KD * P],
            )
            w1_t[i] = t

        PREF = 6
        for i in range(PREF):
            issue_w1(i)
        b1_t = bpool.tile([P, 2 * NF], F32, tag="b1", name="b1")
        nc.scalar.copy(b1_t[:], b1_raw[:])

        gelu = mybir.ActivationFunctionType.Gelu
        h_t = [[[None] * len(tts[c]) for _ in range(NF)] for c in range(2)]

        # ---- G1: chunk A then chunk B, f-major, token tile innermost
        for c in range(2):
            for f in range(NF):
                i = c * NF + f
                pts = [pspool.tile([P, T_TILE], F32, tag=f"ps{ti}",
                                   name=f"p1_{i}_{ti}")
                       for ti in range(len(tts[c]))]
                for k in range(KD):
                    for ti, (off, tsz) in enumerate(tts[c]):
                        nc.tensor.matmul(
                            pts[ti][:, :tsz],
                            w1_t[i][:, k * P:(k + 1) * P],
                            x_t[c][ti][:, k, :],
                            start=(k == 0),
                            stop=(k == KD - 1),
                        )
                for ti, (off, tsz) in enumerate(tts[c]):
                    h = hpool.tile([P, tsz], BF16, tag=f"h{c}_{f}_{ti}",
                                   name=f"h{c}_{f}_{ti}")
                    nc.scalar.activation(h[:], pts[ti][:, :tsz], gelu,
                                         bias=b1_t[:, i:i + 1])
                    h_t[c][f][ti] = h
                if i + PREF < NW1:
                    issue_w1(i + PREF)
                if c == 1 and f in (6, 16, 26):
                    # first w2 slabs, spaced out mid-G1 so they don't compete
                    # with the startup-critical x/w1 transfers
                    issue_slab({6: 0, 16: 1, 26: 2}[f])

        # ---- G2: chunk B first, then chunk A with tiles reversed so the
        # final PSUM->SBUF->DRAM drain is the smallest tile
        for gi, c in enumerate((1, 0)):
            tiles = list(enumerate(tts[c]))
            for dd in range(ND):
                pos = gi * ND + dd
                pts = [pspool.tile([P, T_TILE], F32, tag=f"ps{ti}",
                                   name=f"p2_{c}_{dd}_{ti}")
                       for ti in range(len(tts[c]))]
                for f in range(NF):
                    for ti, (off, tsz) in tiles:
                        nc.tensor.matmul(
                            pts[ti][:, :tsz],
                            slab_t[(c, dd)][:, f * P:(f + 1) * P],
                            h_t[c][f][ti][:],
                            start=(f == 0),
                            stop=(f == NF - 1),
                        )
                out_tiles = tiles if c == 1 else list(reversed(tiles))
                for ti, (off, tsz) in out_tiles:
                    y_t = ypool.tile([P, T_TILE], F32, tag=f"y{ti}",
                                     name=f"y{c}_{dd}_{ti}")
                    nc.vector.tensor_copy(y_t[:, :tsz], pts[ti][:, :tsz])
                    nc.scalar.dma_start(
                        y_h[dd * P:(dd + 1) * P,
                            ybase[c] + off:ybase[c] + off + tsz],
                        y_t[:, :tsz])
                if pos + 3 < NSLAB:
                    issue_slab(pos + 3)
    return _patch_to_json(nc)


def _pack_w1(w1e):
    return (
        w1e.reshape(KD, P, NG, FG, P)
        .transpose(2, 1, 3, 0, 4)
        .reshape(NG * P, FG * KD * P)
    )


def _pack_xs(xf, idx, C, tts):
    xg = np.zeros((C, D), np.float32)
    if len(idx):
        xg[:len(idx)] = xf[idx]
    xgT = np.ascontiguousarray(xg.T).reshape(KD, P, C)
    parts = [
        np.ascontiguousarray(xgT[:, :, off:off + tsz].transpose(1, 0, 2))
        .reshape(P, KD * tsz)
        for off, tsz in tts
    ]
    return np.concatenate(parts, axis=1)


def _kernel_v3(xf, idx_e, wts_e, cnts, w1, b1, w2, b2, C_a, C_b, _trace):
    order = np.argsort(-np.asarray(cnts), kind="stable")
    tts = [_t_tiles_v3a(C_a), _t_tiles_v3b(C_b)]

    key = ("v3", C_a, C_b)
    if key in _neff_cache:
        nc = _neff_cache[key]
    else:
        nc = _build_bass_v3(C_a, C_b)
        _neff_cache[key] = nc

    # core 2i: first halves of (big_i, small_i); core 2i+1: second halves
    core_chunks = []  # per core: [(expert, tok_idx, tok_wts), (chunk B...)]
    for i in range(4):
        be, se = int(order[i]), int(order[7 - i])
        hb = (cnts[be] + 1) // 2
        hs = (cnts[se] + 1) // 2
        core_chunks.append([
            (be, idx_e[be][:hb], wts_e[be][:hb]),
            (se, idx_e[se][:hs], wts_e[se][:hs]),
        ])
        core_chunks.append([
            (be, idx_e[be][hb:], wts_e[be][hb:]),
            (se, idx_e[se][hs:], wts_e[se][hs:]),
        ])

    in_maps = []
    for chunks in core_chunks:
        (be, ia, _), (se, ib, _) = chunks
        xs = np.concatenate(
            [_pack_xs(xf, ia, C_a, tts[0]), _pack_xs(xf, ib, C_b, tts[1])],
            axis=1)
        w1x = np.vstack([_pack_w1(w1[be]), _pack_w1(w1[se])])
        w2x = np.vstack([
            w2[e].reshape(NF, P, ND, P).transpose(2, 1, 0, 3).reshape(ND * P, NF * P)
            for e in (be, se)
        ])
        b1x = np.concatenate(
            [b1[be].reshape(NF, P).T, b1[se].reshape(NF, P).T], axis=1)
        in_maps.append({
            "xs": np.ascontiguousarray(xs).astype(NP_BF16),
            "w1": np.ascontiguousarray(w1x).astype(NP_BF16),
            "b1": np.ascontiguousarray(b1x).astype(np.float32),
            "w2": np.ascontiguousarray(w2x).astype(NP_BF16),
        })

    res = run_bass_kernel_spmd(nc, in_maps, core_ids=list(range(E)), trace=_trace)
    if _trace:
        print(f"HW exec time: {res.exec_time_ns} ns")

    N = xf.shape[0]
    out = np.zeros((N, D), np.float32)
    for core, chunks in enumerate(core_chunks):
        yb = res.results[core]["y"]  # [D, C_a + C_b] f32
        for c, (e, idx, wts) in enumerate(chunks):
            if not len(idx):
                continue
            base = 0 if c == 0 else C_a
            yv = yb[:, base:base + len(idx)].T + b2[e][None, :].astype(np.float32)
            out[idx] += wts[:, None] * yv
    return out


# ---------------------------------------------------------------------------
# v1 fallback builder (handles C > V2_MAX_C; 128-padded capacity, t-tiled)
# ---------------------------------------------------------------------------

def _t_tiles_v1(C):
    out, t0 = [], 0
    while t0 < C:
        tsz = min(T_TILE, C - t0)
        out.append((t0, tsz))
        t0 += tsz
    return out


def _build_bass_v1(C: int) -> bass.Bass:
    nc = bass.Bass()
    tt = _t_tiles_v1(C)
    n_t = len(tt)
    xs_h = nc.dram_tensor("xs", [n_t * P, KD * T_TILE], BF16, kind="ExternalInput")
    w1_h = nc.dram_tensor("w1", [NG * P, KD * FG * P], BF16, kind="ExternalInput")
    b1_h = nc.dram_tensor("b1", [DFF], F32, kind="ExternalInput")
    w2_h = nc.dram_tensor("w2", [DFF, D], BF16, kind="ExternalInput")
    y_h = nc.dram_tensor("y", [ND * n_t * P, T_TILE], F32, kind="ExternalOutput")

    with ExitStack() as ctx:
        tc = ctx.enter_context(tile.TileContext(nc))
        wpool = ctx.enter_context(tc.tile_pool(name="w", bufs=1))
        xpool = ctx.enter_context(tc.tile_pool(name="x", bufs=1))
        hpool = ctx.enter_context(tc.tile_pool(name="h", bufs=1))
        bpool = ctx.enter_context(tc.tile_pool(name="b", bufs=1))
        ypool = ctx.enter_context(tc.tile_pool(name="y", bufs=3))
        ps1 = ctx.enter_context(tc.tile_pool(name="ps1", bufs=3, space="PSUM"))
        ps2 = ctx.enter_context(tc.tile_pool(name="ps2", bufs=3, space="PSUM"))

        BANDS = 8
        BP = P // BANDS
        x_t = [None] * n_t
        for ti, (t0, tsz) in enumerate(tt):
            t = xpool.tile([P, KD, T_TILE], BF16, tag=f"x{ti}", name=f"x{ti}")
            for b in range(BANDS):
                r0 = ti * P + b * BP
                nc.sync.dma_start(
                    t[b * BP:(b + 1) * BP, :, :],
                    xs_h[r0:r0 + BP, :].rearrange("p (kd c) -> p kd c", kd=KD),
                )
            x_t[ti] = t
            if ti == 0:
                w1_t = []
                for g in range(NG):
                    t = wpool.tile([P, KD, FG * P], BF16, tag=f"w1_{g}", name=f"w1_{g}")
                    for b in range(4):
                        r0 = g * P + b * 32
                        nc.sync.dma_start(
                            t[b * 32:(b + 1) * 32, :, :],
                            w1_h[r0:r0 + 32, :].rearrange(
                                "p (kd m) -> p kd m", kd=KD),
                        )
                    w1_t.append(t)
        w2_t = []
        for f in range(NF):
            t = wpool.tile([P, D], BF16, tag=f"w2_{f}", name=f"w2_{f}")
            nc.sync.dma_start(t[:], w2_h[f * P:(f + 1) * P, :])
            w2_t.append(t)
        b1_raw = bpool.tile([P, NF], F32)
        nc.gpsimd.dma_start(b1_raw[:], b1_h[:].rearrange("(f p) -> p f", p=P))
        b1_t = bpool.tile([P, NF], F32)
        nc.scalar.copy(b1_t[:], b1_raw[:])

        gelu = mybir.ActivationFunctionType.Gelu
        for ti, (t0, tsz) in enumerate(tt):
            h_t = [hpool.tile([P, T_TILE], BF16, tag=f"h{f}", name=f"h{f}")
                   for f in range(NF)]
            for f in range(NF):
                pt = ps1.tile([P, T_TILE], F32, tag="ps1", name="pt1")
                for k in range(KD):
                    nc.tensor.matmul(
                        pt[:, :tsz],
                        w1_t[f // FG][:, k, (f % FG) * P:(f % FG + 1) * P],
                        x_t[ti][:, k, :tsz],
                        start=(k == 0),
                        stop=(k == KD - 1),
                    )
                nc.scalar.activation(
                    h_t[f][:, :tsz], pt[:, :tsz], gelu, bias=b1_t[:, f:f + 1]
                )
            for dd in range(ND):
                pt2 = ps2.tile([P, T_TILE], F32, tag="ps2", name="pt2")
                for f in range(NF):
                    nc.tensor.matmul(
                        pt2[:, :tsz],
                        w2_t[f][:, dd * P:(dd + 1) * P],
                        h_t[f][:, :tsz],
                        start=(f == 0),
                        stop=(f == NF - 1),
                    )
                y_t = ypool.tile([P, T_TILE], F32, tag="y", name="yt")
                nc.vector.tensor_copy(y_t[:, :tsz], pt2[:, :tsz])
                r0 = (dd * n_t + ti) * P
                nc.sync.dma_start(y_h[r0:r0 + P, :tsz], y_t[:, :tsz])
    return _patch_to_json(nc)


def _route(xf: np.ndarray, gate_w: np.ndarray):
    """Top-2 gating identical to the reference (argmax ties -> lower index)."""
    N = xf.shape[0]
    logits = xf @ gate_w  # (N, E) f32
    rows = np.arange(N)
    i1 = logits.argmax(1)
    v1 = logits[rows, i1]
    masked = logits.copy()
    masked[rows, i1] = -np.inf
    i2 = masked.argmax(1)
    v2 = masked[rows, i2]
    # softmax over the two selected logits (v1 >= v2)
    e = np.exp((v2 - v1).astype(np.float32))
    wt1 = (1.0 / (1.0 + e)).astype(np.float32)
    wt2 = (e / (1.0 + e)).astype(np.float32)
    idx_e, wts_e = [], []
    for ex in range(E):
        s1 = np.nonzero(i1 == ex)[0]
        s2 = np.nonzero(i2 == ex)[0]
        idx_e.append(np.concatenate([s1, s2]))
        wts_e.append(np.concatenate([wt1[s1], wt2[s2]]).astype(np.float32))
    return idx_e, wts_e


def _kernel_v2(xf, idx_e, wts_e, cnts, w1, b1, w2, b2, C, _trace):
    tt = _t_tiles_v2(C)

    key = ("v2", C)
    if key in _neff_cache:
        nc = _neff_cache[key]
    else:
        nc = _build_bass_v2(C)
        _neff_cache[key] = nc

    in_maps = []
    for ex in range(E):
        cnt = cnts[ex]
        xg = np.zeros((C, D), np.float32)
        if cnt:
            xg[:cnt] = xf[idx_e[ex]]
        xgT = np.ascontiguousarray(xg.T).reshape(KD, P, C)
        parts = [
            np.ascontiguousarray(xgT[:, :, off:off + tsz].transpose(1, 0, 2))
            .reshape(P, KD * tsz)
            for off, tsz in tt
        ]
        xs = np.concatenate(parts, axis=1)
        w1x = (
            w1[ex]
            .reshape(KD, P, NG, FG, P)
            .transpose(2, 1, 3, 0, 4)
            .reshape(NG * P, FG * KD * P)
        )
        in_maps.append({
            "xs": np.ascontiguousarray(xs).astype(NP_BF16),
            "w1": np.ascontiguousarray(w1x).astype(NP_BF16),
            "b1": np.ascontiguousarray(b1[ex].reshape(NF, P).T).astype(np.float32),
            "w2": np.ascontiguousarray(w2[ex]).astype(NP_BF16),
        })

    res = run_bass_kernel_spmd(nc, in_maps, core_ids=list(range(E)), trace=_trace)
    if _trace:
        print(f"HW exec time: {res.exec_time_ns} ns")

    N = xf.shape[0]
    out = np.zeros((N, D), np.float32)
    for ex in range(E):
        cnt = cnts[ex]
        if not cnt:
            continue
        yb = res.results[ex]["y"]  # [D, C] f32
        yv = yb[:, :cnt].T + b2[ex][None, :].astype(np.float32)
        out[idx_e[ex]] += wts_e[ex][:, None] * yv
    return out


def _kernel_v1(xf, idx_e, wts_e, cnts, w1, b1, w2, b2, _trace):
    C = max(P, int(math.ceil(max(cnts) / P)) * P)
    tt = _t_tiles_v1(C)
    n_t = len(tt)
    C_pad = n_t * T_TILE

    key = ("v1", C)
    if key in _neff_cache:
        nc = _neff_cache[key]
    else:
        nc = _build_bass_v1(C)
        _neff_cache[key] = nc

    in_maps = []
    for ex in range(E):
        cnt = cnts[ex]
        xg = np.zeros((C_pad, D), np.float32)
        if cnt:
            xg[:cnt] = xf[idx_e[ex]]
        xs = (
            xg.T.reshape(KD, P, n_t, T_TILE)
            .transpose(2, 1, 0, 3)
            .reshape(n_t * P, KD * T_TILE)
        )
        w1x = (
            w1[ex]
            .reshape(KD, P, NG, FG * P)
            .transpose(2, 1, 0, 3)
            .reshape(NG * P, KD * FG * P)
        )
        in_maps.append({
            "xs": np.ascontiguousarray(xs).astype(NP_BF16),
            "w1": np.ascontiguousarray(w1x).astype(NP_BF16),
            "b1": np.ascontiguousarray(b1[ex]).astype(np.float32),
            "w2": np.ascontiguousarray(w2[ex]).astype(NP_BF16),
        })

    res = run_bass_kernel_spmd(nc, in_maps, core_ids=list(range(E)), trace=_trace)
    if _trace:
        print(f"HW exec time: {res.exec_time_ns} ns")

    N = xf.shape[0]
    out = np.zeros((N, D), np.float32)
    for ex in range(E):
        cnt = cnts[ex]
        if not cnt:
            continue
        yb = res.results[ex]["y"]
        yt = (
            yb.reshape(ND, n_t, P, T_TILE)
            .transpose(0, 2, 1, 3)
            .reshape(D, C_pad)
        )
        yv = yt[:, :cnt].T + b2[ex][None, :].astype(np.float32)
        out[idx_e[ex]] += wts_e[ex][:, None] * yv
    return out


def kernel(x, gate_w, w1, b1, w2, b2, _trace=False):
    B, T, D_ = x.shape
    N = B * T
    xf = np.ascontiguousarray(x.reshape(N, D_).astype(np.float32))
    idx_e, wts_e = _route(xf, gate_w.astype(np.float32))
    cnts = [len(i) for i in idx_e]
    C = max(8, int(math.ceil(max(cnts) / 4)) * 4)

    srt = sorted(cnts, reverse=True)
    C_a = max(8, int(math.ceil(((srt[0] + 1) // 2) / 4)) * 4)
    C_b = max(8, int(math.ceil(((srt[4] + 1) // 2) / 4)) * 4)

    if len(cnts) == E and C_a <= V3_MAX_CA and C_b <= V3_MAX_CB:
        out = _kernel_v3(xf, idx_e, wts_e, cnts, w1, b1, w2, b2, C_a, C_b, _trace)
    elif C <= V2_MAX_C:
        out = _kernel_v2(xf, idx_e, wts_e, cnts, w1, b1, w2, b2, C, _trace)
    else:
        out = _kernel_v1(xf, idx_e, wts_e, cnts, w1, b1, w2, b2, _trace)
    return out.reshape(B, T, D_)
